# revision 1
# baseline (speedup 1.0000x reference)
"""Trainium2 Bass kernel for causal self-attention with RoPE.

Problem: y = CausalSelfAttention(x) with
  B, T, C, H = 4, 2048, 1024, 16; D = 64; RoPE base 10000; no 1/sqrt(D) scale.

Sharding: Megatron-style tensor parallel over heads. 8 cores, 2 heads each.
Each core computes qkv for its 2 heads (columns of W_qkv), runs attention for
its (b, head) pairs, and multiplies by its 128 rows of W_out, producing a
partial (B*T, C) output. The host sums the 8 partials and adds b_out.

Per-core device pipeline:
  phase 1: qT/kT/vT [128, B*T] (2 heads stacked on partitions) via PE matmuls
           with W chunks stationary; RoPE on q/k via a +-1 permutation matmul
           (rotate_half) + DVE mul/adds; v transposed back to [tokens, 128]
           via PE transpose, stored chunked with an interleaved ones column.
  phase 2: per (b, head): S^T = kT_chunk^T @ qT_block on PE (s on partitions),
           exp on ACT (no max subtraction: |score| <~ 60, fp32 exp can't
           overflow), causal mask via 0/1 multiply, P^T @ [v | 1] accumulated
           on PE -> O^T rows + denominator row, normalize via reciprocal +
           K=1 broadcast matmul.
  phase 3: out_partial[tokens, C] = O^T.T @ W_out_rows on PE.

Matmul dtype strategy (RMODE):
  "all":  every matmul in float32r (1 cy/row on PE vs 4 for float32;
          ~12-bit mantissa operand rounding, fp32 accumulate).
  "qk32": q/k projection + scores matmul in float32 (full precision on the
          exp-amplified path), everything else float32r.
  "fp32": everything float32.
"""

import numpy as np
from contextlib import ExitStack

import concourse.mybir as mybir
import concourse.tile as tile
from concourse import bacc
from concourse.bass_utils import run_bass_kernel_spmd
from concourse.masks import make_identity

F32 = mybir.dt.float32
F32R = mybir.dt.float32r
AF = mybir.ActivationFunctionType

C = 1024
H = 16
D = 64
N_CORES = 8
HPC = H // N_CORES          # heads per core = 2
ROPE_BASE = 10000.0
KC = C // 128               # contraction chunks for the qkv projection = 8

RMODE = "all"


def build_program(B, T, use_qk_bias, use_v_bias, rmode=RMODE, n_cores=N_CORES):
    TOK = B * T
    NB = TOK // 512           # 512-token blocks
    NCHUNK = TOK // 128       # 128-token chunks (v storage)
    QB = T // 512             # q-blocks per sequence
    CS = T // 512             # distinct 512-col blocks of the rope tables

    # rmode: "all" | "qk32" | "fp32", or a 3-tuple of dtypes
    # (d_qkproj, d_attn, d_out) for bisection.
    if isinstance(rmode, tuple):
        QKD, AD, OD = rmode
    elif rmode == "all":
        QKD = AD = OD = F32R
    elif rmode == "qk32":
        QKD, AD, OD = F32, F32R, F32R
    else:
        QKD = AD = OD = F32

    nc = bacc.Bacc("TRN2", target_bir_lowering=False, debug=False,
                   num_devices=n_cores)

    xT = nc.dram_tensor("xT", [C, TOK], QKD, kind="ExternalInput").ap()
    wq = nc.dram_tensor("wq", [C, 128], QKD, kind="ExternalInput").ap()
    wk = nc.dram_tensor("wk", [C, 128], QKD, kind="ExternalInput").ap()
    wv = nc.dram_tensor("wv", [C, 128], AD, kind="ExternalInput").ap()
    wo = nc.dram_tensor("wo", [128, C], OD, kind="ExternalInput").ap()
    cosT = nc.dram_tensor("cosT", [128, T], F32, kind="ExternalInput").ap()
    sinT = nc.dram_tensor("sinT", [128, T], F32, kind="ExternalInput").ap()
    msk = nc.dram_tensor("msk", [128, 2048], F32, kind="ExternalInput").ap()
    rot = nc.dram_tensor("rot", [128, 128], AD, kind="ExternalInput").ap()
    if use_qk_bias:
        bq = nc.dram_tensor("bq", [128, 1], F32, kind="ExternalInput").ap()
        bk = nc.dram_tensor("bk", [128, 1], F32, kind="ExternalInput").ap()
    if use_v_bias:
        bv = nc.dram_tensor("bv", [128, 1], F32, kind="ExternalInput").ap()
    outp = nc.dram_tensor("outp", [TOK, C], F32, kind="ExternalOutput").ap()

    with tile.TileContext(nc) as tc:
        with ExitStack() as res:  # tensors resident through phases 1+2
            persist = res.enter_context(tc.tile_pool(name="persist", bufs=1))
            qT = persist.tile([128, TOK], AD)
            kT = persist.tile([128, TOK], AD)
            vsb = persist.tile([128, NCHUNK * 130], AD)
            ones_sb = persist.tile([1, 64], AD)
            nc.vector.memset(ones_sb[:].bitcast(F32) if AD == F32R else ones_sb[:], 1.0)

            with tc.tile_pool(name="ot", bufs=1) as otpool:
                OT = otpool.tile([128, TOK], OD)

                # ---------------- phase 1: qkv projection + RoPE ----------
                with ExitStack() as p1:
                    cpool = p1.enter_context(tc.tile_pool(name="p1c", bufs=1))
                    wq_sb = cpool.tile([128, C], QKD)
                    wk_sb = cpool.tile([128, C], QKD)
                    wv_sb = cpool.tile([128, C], AD)
                    cos_sb = cpool.tile([128, T], F32)
                    sin_sb = cpool.tile([128, T], F32)
                    rot_sb = cpool.tile([128, 128], AD)
                    ident = cpool.tile([128, 128], F32)
                    make_identity(nc, ident[:])
                    for k in range(KC):
                        nc.sync.dma_start(wq_sb[:, k * 128:(k + 1) * 128],
                                          wq[k * 128:(k + 1) * 128, :])
                        nc.sync.dma_start(wk_sb[:, k * 128:(k + 1) * 128],
                                          wk[k * 128:(k + 1) * 128, :])
                        nc.sync.dma_start(wv_sb[:, k * 128:(k + 1) * 128],
                                          wv[k * 128:(k + 1) * 128, :])
                    nc.sync.dma_start(cos_sb[:], cosT[:])
                    nc.sync.dma_start(sin_sb[:], sinT[:])
                    nc.sync.dma_start(rot_sb[:], rot[:])
                    if use_qk_bias:
                        bq_sb = cpool.tile([128, 1], F32)
                        bk_sb = cpool.tile([128, 1], F32)
                        nc.sync.dma_start(bq_sb[:], bq[:])
                        nc.sync.dma_start(bk_sb[:], bk[:])
                    if use_v_bias:
                        bv_sb = cpool.tile([128, 1], F32)
                        nc.sync.dma_start(bv_sb[:], bv[:])

                    xpool = p1.enter_context(tc.tile_pool(name="xp", bufs=8))
                    xrpool = (xpool if QKD == AD else
                              p1.enter_context(tc.tile_pool(name="xrp", bufs=6)))
                    qkpsum = p1.enter_context(
                        tc.tile_pool(name="qkp", bufs=2, space="PSUM"))
                    rotpsum = p1.enter_context(
                        tc.tile_pool(name="rotp", bufs=1, space="PSUM"))
                    vpsum = p1.enter_context(
                        tc.tile_pool(name="vp", bufs=2, space="PSUM"))
                    tppsum = p1.enter_context(
                        tc.tile_pool(name="tpp", bufs=2, space="PSUM"))
                    tmp = p1.enter_context(tc.tile_pool(name="tmp", bufs=6))

                    for nb in range(NB):
                        t512 = slice(nb * 512, (nb + 1) * 512)
                        cs = slice((nb % CS) * 512, (nb % CS) * 512 + 512)
                        xc = []
                        for k in range(KC):
                            t = xpool.tile([128, 512], QKD, tag="xc")
                            nc.sync.dma_start(
                                t[:], xT[k * 128:(k + 1) * 128, t512])
                            xc.append(t)
                        if QKD == AD:
                            xcv = xc
                        else:
                            # round a second copy for the fp32r v projection
                            xcv = []
                            for k in range(KC):
                                t = xrpool.tile([128, 512], AD, tag="xcv")
                                nc.gpsimd.tensor_copy(t[:], xc[k][:])
                                xcv.append(t)
                        for w_sb, b_name, dstT in ((wq_sb, "bq", qT),
                                                   (wk_sb, "bk", kT)):
                            acc = qkpsum.tile([128, 512], F32, tag="acc")
                            for k in range(KC):
                                nc.tensor.matmul(
                                    acc[:], w_sb[:, k * 128:(k + 1) * 128],
                                    xc[k][:], start=(k == 0), stop=(k == KC - 1))
                            raw = tmp.tile([128, 512], AD, tag="ropetmp")
                            if use_qk_bias:
                                b_sb = bq_sb if b_name == "bq" else bk_sb
                                nc.vector.tensor_scalar_add(raw[:], acc[:],
                                                            b_sb[:])
                            else:
                                nc.vector.tensor_copy(raw[:], acc[:])
                            rp = rotpsum.tile([128, 512], F32, tag="rp")
                            nc.tensor.matmul(rp[:], rot_sb[:], raw[:],
                                             start=True, stop=True)
                            t1 = tmp.tile([128, 512], F32, tag="ropetmp")
                            nc.vector.tensor_mul(t1[:], raw[:], cos_sb[:, cs])
                            t2 = tmp.tile([128, 512], F32, tag="ropetmp")
                            nc.vector.tensor_mul(t2[:], rp[:], sin_sb[:, cs])
                            nc.vector.tensor_add(dstT[:, t512], t1[:], t2[:])
                        # v: project transposed (N=512 streams), then PE-
                        # transpose back to [tokens, 128] chunks
                        vacc = vpsum.tile([128, 512], F32, tag="vacc")
                        for k in range(KC):
                            nc.tensor.matmul(
                                vacc[:], wv_sb[:, k * 128:(k + 1) * 128],
                                xcv[k][:], start=(k == 0), stop=(k == KC - 1))
                        vraw = tmp.tile([128, 512], F32, tag="vraw", bufs=2)
                        if use_v_bias:
                            nc.vector.tensor_scalar_add(vraw[:], vacc[:],
                                                        bv_sb[:])
                        else:
                            nc.vector.tensor_copy(vraw[:], vacc[:])
                        for sub in range(4):
                            tp = tppsum.tile([128, 128], F32, tag="tp")
                            nc.tensor.transpose(
                                tp[:], vraw[:, sub * 128:(sub + 1) * 128],
                                ident[:])
                            base = (nb * 4 + sub) * 130
                            nc.vector.tensor_copy(vsb[:, base:base + 64],
                                                  tp[:, 0:64])
                            nc.vector.tensor_copy(vsb[:, base + 65:base + 129],
                                                  tp[:, 64:128])
                            nc.vector.memset(vsb[:, base + 64:base + 65].bitcast(F32) if AD == F32R else vsb[:, base + 64:base + 65], 1.0)
                            nc.vector.memset(vsb[:, base + 129:base + 130].bitcast(F32) if AD == F32R else vsb[:, base + 129:base + 130], 1.0)

                # ---------------- phase 2: causal attention ---------------
                with ExitStack() as p2:
                    mpool = p2.enter_context(tc.tile_pool(name="mp", bufs=1))
                    msk_sb = mpool.tile([128, 2048], F32)
                    nc.sync.dma_start(msk_sb[:], msk[:])
                    spsum = p2.enter_context(
                        tc.tile_pool(name="sp", bufs=3, space="PSUM"))
                    popsum = p2.enter_context(
                        tc.tile_pool(name="pop", bufs=2, space="PSUM"))
                    bcpsum = p2.enter_context(
                        tc.tile_pool(name="bcp", bufs=2, space="PSUM"))
                    ppool = p2.enter_context(tc.tile_pool(name="pp", bufs=3))
                    dpool = p2.enter_context(tc.tile_pool(name="dp", bufs=2))
                    bspool = p2.enter_context(tc.tile_pool(name="bs", bufs=2))

                    for b in range(B):
                        t0 = b * T
                        c0 = b * (T // 128)
                        for hp in range(HPC):
                            hs = slice(hp * 64, (hp + 1) * 64)
                            for qb in range(QB):
                                q512 = slice(t0 + qb * 512, t0 + (qb + 1) * 512)
                                po = popsum.tile([65, 512], F32, tag="po")
                                ns = (qb + 1) * 4
                                for si in range(ns):
                                    S = spsum.tile([128, 512], F32, tag="S")
                                    s0 = t0 + si * 128
                                    nc.tensor.matmul(
                                        S[:], kT[hs, s0:s0 + 128],
                                        qT[hs, q512], start=True, stop=True)
                                    P = ppool.tile([128, 512], AD, tag="P")
                                    nc.scalar.activation(P[:], S[:], AF.Exp)
                                    off = si * 128 - qb * 512
                                    if off >= 0:
                                        vi = off // 128
                                        nc.vector.tensor_mul(
                                            P[:], P[:],
                                            msk_sb[:, vi * 512:(vi + 1) * 512])
                                    vbase = (c0 + si) * 130 + hp * 65
                                    nc.tensor.matmul(
                                        po[:], vsb[:, vbase:vbase + 65], P[:],
                                        start=(si == 0), stop=(si == ns - 1))
                                den32 = dpool.tile([1, 512], F32, tag="den32")
                                nc.vector.reciprocal(den32[:], po[64:65, :])
                                den = dpool.tile([1, 512], AD, tag="den")
                                nc.vector.tensor_copy(den[:], den32[:])
                                bc = bcpsum.tile([64, 512], F32, tag="bc")
                                nc.tensor.matmul(bc[:], ones_sb[:], den[:],
                                                 start=True, stop=True)
                                bcs = bspool.tile([64, 512], F32, tag="bcs")
                                nc.vector.tensor_copy(bcs[:], bc[:])
                                nc.vector.tensor_mul(
                                    OT[hs, q512], po[0:64, :], bcs[:])

                # ---------------- phase 3: output projection --------------
                with ExitStack() as p3:
                    wpool = p3.enter_context(tc.tile_pool(name="wop", bufs=1))
                    wo_sb = wpool.tile([128, C], OD)
                    nc.sync.dma_start(wo_sb[:], wo[:])
                    opsum = p3.enter_context(
                        tc.tile_pool(name="op", bufs=4, space="PSUM"))
                    ostage = p3.enter_context(tc.tile_pool(name="os", bufs=4))
                    for ci in range(NCHUNK):
                        for n2 in range(C // 512):
                            pacc = opsum.tile([128, 512], F32, tag="pacc")
                            nc.tensor.matmul(
                                pacc[:], OT[:, ci * 128:(ci + 1) * 128],
                                wo_sb[:, n2 * 512:(n2 + 1) * 512],
                                start=True, stop=True)
                            osb = ostage.tile([128, 512], F32, tag="osb")
                            if (ci + n2) % 2 == 0:
                                nc.scalar.activation(osb[:], pacc[:], AF.Copy)
                            else:
                                nc.vector.tensor_copy(osb[:], pacc[:])
                            nc.sync.dma_start(
                                outp[ci * 128:(ci + 1) * 128,
                                     n2 * 512:(n2 + 1) * 512], osb[:])

    nc.compile()
    return nc


def make_rope_tables(T, dtype=np.float32):
    j = np.arange(32, dtype=np.float32)
    inv_freq = (1.0 / (ROPE_BASE ** (2.0 * j / D))).astype(np.float32)
    t = np.arange(T, dtype=np.float32)
    freqs = t[None, :] * inv_freq[:, None]          # [32, T]
    half = np.concatenate([freqs, freqs], axis=0)   # [64, T]
    cosT = np.cos(half).astype(dtype)
    sinT = np.sin(half).astype(dtype)
    return (np.concatenate([cosT, cosT], axis=0),   # [128, T] (2 heads)
            np.concatenate([sinT, sinT], axis=0))


def make_rot_matrix():
    """lhsT R [128,128] s.t. (R.T @ x)[m] = rotate_half(x)[m] per 64-row head."""
    R = np.zeros((128, 128), dtype=np.float32)
    for hb in (0, 64):
        for m in range(32):
            R[hb + m + 32, hb + m] = -1.0
            R[hb + m, hb + m + 32] = 1.0
    return R


def make_masks():
    """[128, 4*512]: variant vi valid where s + vi*128 <= q."""
    s = np.arange(128)[:, None]
    q = np.arange(512)[None, :]
    blocks = [(s + vi * 128 <= q).astype(np.float32) for vi in range(4)]
    return np.concatenate(blocks, axis=1)


def prep_in_maps(x, W_qkv, b_qkv, W_out, B, T, use_qk_bias, use_v_bias,
                 n_cores=N_CORES):
    TOK = B * T
    xTm = np.ascontiguousarray(x.reshape(TOK, C).T)
    cosT, sinT = make_rope_tables(T)
    msk = make_masks()
    rot = make_rot_matrix()
    in_maps = []
    for c in range(n_cores):
        h0 = c * HPC
        cols = slice(h0 * D, (h0 + HPC) * D)        # 128 head-dim columns
        m = {
            "xT": xTm,
            "wq": np.ascontiguousarray(W_qkv[:, cols]),
            "wk": np.ascontiguousarray(W_qkv[:, C:][:, cols]),
            "wv": np.ascontiguousarray(W_qkv[:, 2 * C:][:, cols]),
            "wo": np.ascontiguousarray(W_out[cols, :]),
            "cosT": cosT, "sinT": sinT, "msk": msk, "rot": rot,
        }
        if use_qk_bias:
            m["bq"] = np.ascontiguousarray(b_qkv[cols]).reshape(128, 1)
            m["bk"] = np.ascontiguousarray(b_qkv[C:][cols]).reshape(128, 1)
        if use_v_bias:
            m["bv"] = np.ascontiguousarray(b_qkv[2 * C:][cols]).reshape(128, 1)
        in_maps.append(m)
    return in_maps


_CACHE = {}


def _get_program(key):
    if key not in _CACHE:
        B, T, use_qk_bias, use_v_bias = key
        _CACHE[key] = build_program(B, T, use_qk_bias, use_v_bias)
    return _CACHE[key]


def kernel(x, W_qkv, b_qkv, W_out, b_out):
    x = np.asarray(x, dtype=np.float32)
    W_qkv = np.asarray(W_qkv, dtype=np.float32)
    b_qkv = np.asarray(b_qkv, dtype=np.float32)
    W_out = np.asarray(W_out, dtype=np.float32)
    b_out = np.asarray(b_out, dtype=np.float32)
    B, T, _ = x.shape
    use_qk_bias = bool(np.any(b_qkv[:2 * C]))
    use_v_bias = bool(np.any(b_qkv[2 * C:]))
    nc = _get_program((B, T, use_qk_bias, use_v_bias))
    in_maps = prep_in_maps(x, W_qkv, b_qkv, W_out, B, T,
                           use_qk_bias, use_v_bias)
    res = run_bass_kernel_spmd(nc, in_maps, list(range(N_CORES)))
    acc = res.results[0]["outp"].astype(np.float32)
    for c in range(1, N_CORES):
        acc = acc + res.results[c]["outp"]
    acc = acc + b_out[None, :]
    return acc.reshape(B, T, C)



# revision 17
# speedup vs baseline: 1.4965x; 1.4965x over previous
"""Trainium2 Bass kernel for causal self-attention with RoPE (v2).

Problem: y = CausalSelfAttention(x) with
  B, T, C, H = 4, 2048, 1024, 16; D = 64; RoPE base 10000; no 1/sqrt(D) scale.

Sharding (hybrid data x tensor parallel): core c -> batch b = c//2, head-half
hh = c%2 (8 heads = 4 head-pairs). Each core computes qkv for its batch over
its 8 heads' weight columns, attention for those heads, and the out-projection
against its 512 rows of W_out, producing a partial [T, C]. The host sums the
2 partials per batch and adds biases. 4x less DMA than pure head sharding.

Per-core pipeline (engine-balanced, PE kept continuously busy for HAM warmth):
  p1 (per pair): qT/kT [128, T] (2 heads on partitions) via PE matmuls; RoPE
      via t2s = raw*sin (DVE, PSUM src), rp = R^T t2s (PE; exploits
      sin[i]==sin[i+32] so R^T(q*sin) == rot_half(q)*sin), q = raw*cos + rp.
      v projected then PE-transposed to [tokens, d] bf16 chunks with a shared
      64-wide ones block: [vA 64 | ones 64 | vB 64] per 128-token chunk.
  p2 (per pair, head, half): s-chunk-outer "strip" softmax. For s-chunk si,
      S strip [128 s, <=1024 q] via K=64 fp32r matmuls, ONE exp over the strip
      (ACT, bf16 out), causal mask only on the 128-wide diagonal block (DVE),
      PV accumulation po[128, 1024] with M=128 stationary [v|ones] so rows
      carry both O (64) and the denominator replicated 64x. Strip loop is
      software-pipelined (PV of strip si-1 emitted after S of si) and p1 work
      of pair p+1 is interleaved at unit granularity so the PE never idles.
  p3: recip = exp(-ln(den)) on ACT (one table set switch, DVE reciprocal is
      8cy/elem and ACT Reciprocal is banned), normalize OT in place (bf16),
      out = OT^T @ W_out accumulated over pairs in PSUM, DMA partial out.
"""

import numpy as np
from contextlib import ExitStack

import ml_dtypes
import concourse.mybir as mybir
import concourse.tile as tile
from concourse import bacc
from concourse.bass_utils import run_bass_kernel_spmd
from concourse.masks import make_identity

# Force Exp/Ln/Copy activations to resolve to the one table set containing
# all of them (natural_log_exp_and_others). Without this, the Tile scheduler
# interleaves p3's Ln with attention Exp on the ACT queue and every
# alternation pays a ~2.7us ACT_TABLE_LOAD + drain, which also starves the
# PE long enough to drop the HAM clock gate to half speed. Set positions are
# preserved so act_func_set_id indices stay valid.
_gat_orig = bacc.get_activation_tables
_UNIFIED = "natural_log_exp_and_others"


def _gat_unified(arch):
    tabs = _gat_orig(arch)
    if _UNIFIED in tabs:
        shared = {f for f in tabs[_UNIFIED]
                  if f.name.lower() in ("exp", "ln", "copy", "identity")}
        tabs = {name: (s if name == _UNIFIED else (s - shared))
                for name, s in tabs.items()}
    return tabs


bacc.get_activation_tables = _gat_unified

F32 = mybir.dt.float32
F32R = mybir.dt.float32r
BF16 = mybir.dt.bfloat16
AF = mybir.ActivationFunctionType

C = 1024
H = 16
D = 64
N_CORES = 8
T = 2048                 # tokens per core (one batch)
NP = 4                   # head pairs per core
KC = C // 128            # 8 contraction chunks for projections
TB = T // 512            # 4 token blocks
NCH = T // 128           # 16 v chunks per pair
VW = 192                 # vsb cols per chunk: [vA 64 | ones 64 | vB 64]
ROPE_BASE = 10000.0
DEBUG = False


def _chop512(a, b):
    """Split [a, b) at 512 boundaries -> list of (lo, hi)."""
    out = []
    while a < b:
        nxt = min(b, (a // 512 + 1) * 512)
        out.append((a, nxt))
        a = nxt
    return out


def build_program(use_qk_bias):
    nc = bacc.Bacc("TRN2", target_bir_lowering=False, debug=False,
                   num_devices=N_CORES)

    xT = nc.dram_tensor("xT", [C, T], F32R, kind="ExternalInput").ap()
    wq = nc.dram_tensor("wq", [C, 512], F32R, kind="ExternalInput").ap()
    wk = nc.dram_tensor("wk", [C, 512], F32R, kind="ExternalInput").ap()
    wv = nc.dram_tensor("wv", [C, 512], F32R, kind="ExternalInput").ap()
    wo = nc.dram_tensor("wo", [512, C], BF16, kind="ExternalInput").ap()
    cosT = nc.dram_tensor("cosT", [128, T], F32, kind="ExternalInput").ap()
    sinT = nc.dram_tensor("sinT", [128, T], F32, kind="ExternalInput").ap()
    mskP = nc.dram_tensor("mskP", [128, 128], BF16, kind="ExternalInput").ap()
    rot = nc.dram_tensor("rot", [128, 128], F32R, kind="ExternalInput").ap()
    if use_qk_bias:
        bq = nc.dram_tensor("bq", [128, NP], F32, kind="ExternalInput").ap()
        bk = nc.dram_tensor("bk", [128, NP], F32, kind="ExternalInput").ap()
    outp = nc.dram_tensor("outp", [T, C], F32, kind="ExternalOutput").ap()
    if DEBUG:
        qTd = nc.dram_tensor("qTd", [128, NP * T], F32, kind="ExternalOutput").ap()
        kTd = nc.dram_tensor("kTd", [128, NP * T], F32, kind="ExternalOutput").ap()
        vsbd = nc.dram_tensor("vsbd", [128, NP * NCH * VW], F32, kind="ExternalOutput").ap()
        OTd = nc.dram_tensor("OTd", [128, NP * T], F32, kind="ExternalOutput").ap()
        densd = nc.dram_tensor("densd", [128, 8 * 1024], F32, kind="ExternalOutput").ap()
        densr = nc.dram_tensor("densr", [128, 8 * 1024], F32, kind="ExternalOutput").ap()
        bcd = nc.dram_tensor("bcd", [128, 16 * 1024], F32, kind="ExternalOutput").ap()
        OTn = nc.dram_tensor("OTn", [128, NP * T], F32, kind="ExternalOutput").ap()

    with tile.TileContext(nc) as tc:
        with ExitStack() as res:
            persist = res.enter_context(tc.tile_pool(name="persist", bufs=1))
            qT = persist.tile([128, NP * T], F32R)
            kT = persist.tile([128, NP * T], F32R)
            vsb = persist.tile([128, NP * NCH * VW], BF16)
            OT = persist.tile([128, NP * T], BF16)
            # den staging: slot (p*2+f) x 1024 cols; row 64 = head A's den,
            # row 0 = head B's den (lane-aligned copies out of po)
            dens = persist.tile([128, 8 * 1024], BF16)
            cos_sb = persist.tile([128, T], F32)
            sin_sb = persist.tile([128, T], F32)
            msk_sb = persist.tile([128, 128], BF16)
            rot_sb = persist.tile([128, 128], F32R)
            ident = persist.tile([128, 128], BF16)
            ones_sb = persist.tile([128, 128], BF16)
            nc.vector.memset(ones_sb[:], 1.0)
            shift_sb = persist.tile([128, 1], F32)
            nc.vector.memset(shift_sb[:], -15.0)
            nc.sync.dma_start(cos_sb[:], cosT[:])
            nc.sync.dma_start(sin_sb[:], sinT[:])
            nc.sync.dma_start(msk_sb[:], mskP[:])
            nc.sync.dma_start(rot_sb[:], rot[:])
            nc.vector.memset(dens[:], 1.0)
            make_identity(nc, ident[:])
            if use_qk_bias:
                bq_sb = persist.tile([128, NP], F32)
                bk_sb = persist.tile([128, NP], F32)
                nc.sync.dma_start(bq_sb[:], bq[:])
                nc.sync.dma_start(bk_sb[:], bk[:])

            with ExitStack() as p12:
                wpool = p12.enter_context(tc.tile_pool(name="wp", bufs=2))
                xpool = p12.enter_context(tc.tile_pool(name="xp", bufs=10))
                stage = p12.enter_context(tc.tile_pool(name="st", bufs=3))
                vrawp = p12.enter_context(tc.tile_pool(name="vr", bufs=2))
                ppool = p12.enter_context(tc.tile_pool(name="pp", bufs=2))
                accP = p12.enter_context(
                    tc.tile_pool(name="accP", bufs=2, space="PSUM"))
                rpP = p12.enter_context(
                    tc.tile_pool(name="rpP", bufs=2, space="PSUM"))
                stripP = p12.enter_context(
                    tc.tile_pool(name="stripP", bufs=1, space="PSUM"))
                poP = p12.enter_context(
                    tc.tile_pool(name="poP", bufs=1, space="PSUM"))

                def p1_units(p):
                    """Projection + rope + v for pair p. Yields per unit."""
                    wq_sb = wpool.tile([128, C], F32R, tag="wq")
                    wk_sb = wpool.tile([128, C], F32R, tag="wk")
                    wv_sb = wpool.tile([128, C], F32R, tag="wv")
                    for k in range(KC):
                        ks = slice(k * 128, (k + 1) * 128)
                        ps = slice(p * 128, (p + 1) * 128)
                        nc.sync.dma_start(wq_sb[:, ks], wq[ks, ps])
                        nc.sync.dma_start(wk_sb[:, ks], wk[ks, ps])
                        nc.sync.dma_start(wv_sb[:, ks], wv[ks, ps])
                    # ones blocks for this pair's v chunks
                    for ch in range(NCH):
                        cb = (p * NCH + ch) * VW
                        nc.vector.memset(vsb[:, cb + 64:cb + 128], 1.0)
                    yield
                    for tb in range(TB):
                        tsl = slice(tb * 512, (tb + 1) * 512)         # tokens
                        dsl = slice(p * T + tb * 512, p * T + (tb + 1) * 512)
                        xc = []
                        for k in range(KC):
                            t = xpool.tile([128, 512], F32R, tag="xc")
                            nc.sync.dma_start(
                                t[:], xT[k * 128:(k + 1) * 128, tsl])
                            xc.append(t)
                        for w_sb, b_name, dstT in ((wq_sb, "bq", qT),
                                                   (wk_sb, "bk", kT)):
                            acc = accP.tile([128, 512], F32, tag="acc")
                            for k in range(KC):
                                nc.tensor.matmul(
                                    acc[:], w_sb[:, k * 128:(k + 1) * 128],
                                    xc[k][:], start=(k == 0), stop=(k == KC - 1))
                            yield
                            if use_qk_bias:
                                b_sb = bq_sb if b_name == "bq" else bk_sb
                                raws = stage.tile([128, 512], F32R, tag="st")
                                nc.vector.tensor_scalar_add(
                                    raws[:], acc[:], b_sb[:, p:p + 1])
                                src = raws
                            else:
                                src = acc
                            t2s = stage.tile([128, 512], F32R, tag="st")
                            nc.vector.tensor_mul(t2s[:], src[:], sin_sb[:, tsl])
                            rp = rpP.tile([128, 512], F32, tag="rp")
                            nc.tensor.matmul(rp[:], rot_sb[:], t2s[:],
                                             start=True, stop=True)
                            t1 = stage.tile([128, 512], F32R, tag="st")
                            nc.vector.tensor_mul(t1[:], src[:], cos_sb[:, tsl])
                            nc.vector.tensor_add(dstT[:, dsl], t1[:], rp[:])
                            yield
                        vacc = accP.tile([128, 512], F32, tag="acc")
                        for k in range(KC):
                            nc.tensor.matmul(
                                vacc[:], wv_sb[:, k * 128:(k + 1) * 128],
                                xc[k][:], start=(k == 0), stop=(k == KC - 1))
                        yield
                        vraw = vrawp.tile([128, 512], BF16, tag="vr")
                        nc.vector.tensor_copy(vraw[:], vacc[:])
                        for sub in range(4):
                            tp = rpP.tile([128, 512], BF16, tag="rp")
                            nc.tensor.transpose(
                                tp[:, 0:128],
                                vraw[:, sub * 128:(sub + 1) * 128], ident[:])
                            cb = (p * NCH + tb * 4 + sub) * VW
                            nc.vector.tensor_copy(vsb[:, cb:cb + 64],
                                                  tp[:, 0:64])
                            nc.vector.tensor_copy(vsb[:, cb + 128:cb + 192],
                                                  tp[:, 64:128])
                        yield

                def p2_units(p):
                    """Attention for pair p (2 heads x 2 halves, strips)."""
                    for h in range(2):
                        hs = slice(h * 64, (h + 1) * 64)
                        for f in range(2):
                            base = f * 1024
                            strips = list(range(8 * f + 8))
                            po = poP.tile([128, 1024], F32, tag="po")
                            prev = None
                            for si in strips + [None]:
                                if si is not None:
                                    qlo = max(si * 128, base)
                                    pieces = _chop512(qlo, base + 1024)
                                    S = stripP.tile([128, 1024], F32, tag="S")
                                    for (a, b) in pieces:
                                        nc.tensor.matmul(
                                            S[:, a - base:b - base],
                                            kT[hs, p * T + si * 128:
                                               p * T + si * 128 + 128],
                                            qT[hs, p * T + a:p * T + b],
                                            start=True, stop=True)
                                    P = ppool.tile([128, 1024], BF16, tag="pp")
                                    cL = qlo - base
                                    # exp(S - SHIFT): keeps den inside ACT
                                    # Ln's valid window (|ln x| < ~45);
                                    # cancels exactly in the normalization.
                                    nc.scalar.activation(P[:, cL:1024],
                                                         S[:, cL:1024],
                                                         AF.Exp,
                                                         bias=shift_sb[:, 0:1])
                                    if si * 128 >= base:   # diagonal block
                                        nc.vector.tensor_mul(
                                            P[:, cL:cL + 128],
                                            P[:, cL:cL + 128], msk_sb[:])
                                    cur = (si, P, pieces)
                                else:
                                    cur = None
                                if prev is not None:
                                    psi, pP, ppieces = prev
                                    vb = (p * NCH + psi) * VW + h * 64
                                    for (a, b) in ppieces:
                                        qb = a // 512
                                        nc.tensor.matmul(
                                            po[:, a - base:b - base],
                                            vsb[:, vb:vb + 128],
                                            pP[:, a - base:b - base],
                                            start=(psi == 0),
                                            stop=(psi == 4 * qb + 3))
                                prev = cur
                                yield
                            # drain: O rows (lane-aligned) + one den row
                            slot = p * 2 + f
                            ssl = slice(slot * 1024, (slot + 1) * 1024)
                            nc.vector.tensor_copy(
                                OT[hs, p * T + base:p * T + base + 1024],
                                po[hs, :])
                            drow = slice(64, 65) if h == 0 else slice(0, 1)
                            nc.vector.tensor_copy(dens[drow, ssl],
                                                  po[drow, :])
                            yield

                # ---- interleaved emission: p1(pair p+1) inside p2(pair p)
                g1 = p1_units(0)
                for _ in g1:
                    pass
                for p in range(NP):
                    g1 = p1_units(p + 1) if p + 1 < NP else None
                    cnt = 0
                    for _ in p2_units(p):
                        cnt += 1
                        if g1 is not None and cnt % 2 == 0:
                            next(g1, None)
                    if g1 is not None:
                        for _ in g1:
                            pass

            # ---------------- p3: normalize + output projection ----------
            with ExitStack() as p3:
                if DEBUG:
                    dbgp = p3.enter_context(tc.tile_pool(name="dbg", bufs=2))
                    for blk in range(NP * T // 512):
                        bsl = slice(blk * 512, (blk + 1) * 512)
                        for name, sb, dr in (("q", qT, qTd), ("k", kT, kTd),
                                             ("o", OT, OTd)):
                            t = dbgp.tile([128, 512], F32, tag="dbg")
                            nc.vector.tensor_copy(t[:], sb[:, bsl])
                            nc.sync.dma_start(dr[:, bsl], t[:])
                    for blk in range(NP * NCH * VW // 512):
                        bsl = slice(blk * 512, (blk + 1) * 512)
                        t = dbgp.tile([128, 512], F32, tag="dbg")
                        nc.vector.tensor_copy(t[:], vsb[:, bsl])
                        nc.sync.dma_start(vsbd[:, bsl], t[:])
                    for blk in range(8 * 1024 // 512):
                        bsl = slice(blk * 512, (blk + 1) * 512)
                        t = dbgp.tile([128, 512], F32, tag="dbg")
                        nc.vector.tensor_copy(t[:], dens[:, bsl])
                        nc.sync.dma_start(densd[:, bsl], t[:])

                wop = p3.enter_context(tc.tile_pool(name="wop", bufs=1))
                lnp = p3.enter_context(tc.tile_pool(name="lnp", bufs=8))
                bcP = p3.enter_context(
                    tc.tile_pool(name="bcP", bufs=2, space="PSUM"))
                outP = p3.enter_context(
                    tc.tile_pool(name="outP", bufs=2, space="PSUM"))
                ostage = p3.enter_context(tc.tile_pool(name="os", bufs=3))
                wo_sb = wop.tile([128, NP * C], BF16)
                for p in range(NP):
                    nc.sync.dma_start(wo_sb[:, p * C:(p + 1) * C],
                                      wo[p * 128:(p + 1) * 128, :])
                # 1/den = exp(-ln(den)); all Ln then all Exp (table sets).
                # Full-height ops: FD-limited anyway, and rows other than
                # 0/64 hold 1.0 from the init memset (ln -> 0, exp -> 1).
                lns = []
                for slot in range(8):
                    ssl = slice(slot * 1024, (slot + 1) * 1024)
                    lt = lnp.tile([128, 1024], F32, tag="ln")
                    nc.scalar.activation(lt[:], dens[:, ssl], AF.Ln)
                    lns.append((lt, ssl))
                for lt, ssl in lns:
                    nc.scalar.activation(dens[:, ssl], lt[:], AF.Exp,
                                         scale=-1.0)
                # broadcast each head's recip row across 128 lanes via a
                # K=1 ones matmul (lhsT row at the SAME partition as the
                # dens row so the auto tile_position stays consistent),
                # then normalize OT in place reading the bc from PSUM.
                for slot in range(8):
                    p, f = slot // 2, slot % 2
                    ssl = slice(slot * 1024, (slot + 1) * 1024)
                    osl = slice(p * T + f * 1024, p * T + f * 1024 + 1024)
                    for h in range(2):
                        hs = slice(h * 64, (h + 1) * 64)
                        drow = slice(64, 65) if h == 0 else slice(0, 1)
                        bc = bcP.tile([128, 1024], F32, tag="bc")
                        for n in range(2):
                            nc.tensor.matmul(
                                bc[:, n * 512:(n + 1) * 512],
                                ones_sb[drow, 0:128],
                                dens[drow, slot * 1024 + n * 512:
                                     slot * 1024 + (n + 1) * 512],
                                start=True, stop=True)
                        if DEBUG:
                            dt_ = dbgp.tile([128, 1024], F32, tag="dbg")
                            nc.vector.tensor_copy(dt_[:], bc[:])
                            nc.sync.dma_start(
                                bcd[:, (slot * 2 + h) * 1024:
                                    (slot * 2 + h + 1) * 1024], dt_[:])
                        nc.vector.tensor_mul(OT[hs, osl], OT[hs, osl],
                                             bc[hs, :])
                if DEBUG:
                    for blk in range(8):
                        bsl = slice(blk * 1024, (blk + 1) * 1024)
                        t = dbgp.tile([128, 1024], F32, tag="dbg")
                        nc.vector.tensor_copy(t[:], dens[:, bsl])
                        nc.sync.dma_start(densr[:, bsl], t[:])
                    for blk in range(NP * T // 1024):
                        bsl = slice(blk * 1024, (blk + 1) * 1024)
                        t = dbgp.tile([128, 1024], F32, tag="dbg")
                        nc.vector.tensor_copy(t[:], OT[:, bsl])
                        nc.sync.dma_start(OTn[:, bsl], t[:])
                for tc_i in range(T // 128):
                    oacc = outP.tile([128, C], F32, tag="oacc")
                    for n in range(2):
                        for p in range(NP):
                            nc.tensor.matmul(
                                oacc[:, n * 512:(n + 1) * 512],
                                OT[:, p * T + tc_i * 128:
                                   p * T + tc_i * 128 + 128],
                                wo_sb[:, p * C + n * 512:p * C + n * 512 + 512],
                                start=(p == 0), stop=(p == NP - 1))
                    osb = ostage.tile([128, C], F32, tag="os")
                    if tc_i % 2 == 0:
                        nc.scalar.activation(osb[:], oacc[:], AF.Copy)
                    else:
                        nc.vector.tensor_copy(osb[:], oacc[:])
                    nc.sync.dma_start(
                        outp[tc_i * 128:(tc_i + 1) * 128, :], osb[:])

    nc.compile()
    return nc


def make_rope_tables(t_len, dtype=np.float32):
    j = np.arange(32, dtype=np.float32)
    inv_freq = (1.0 / (ROPE_BASE ** (2.0 * j / D))).astype(np.float32)
    t = np.arange(t_len, dtype=np.float32)
    freqs = t[None, :] * inv_freq[:, None]          # [32, T]
    half = np.concatenate([freqs, freqs], axis=0)   # [64, T]
    cosT = np.cos(half).astype(dtype)
    sinT = np.sin(half).astype(dtype)
    return (np.concatenate([cosT, cosT], axis=0),   # [128, T] (2 heads)
            np.concatenate([sinT, sinT], axis=0))


def make_rot_matrix():
    """lhsT R [128,128] s.t. (R.T @ x)[m] = rotate_half(x)[m] per 64-row head."""
    R = np.zeros((128, 128), dtype=np.float32)
    for hb in (0, 64):
        for m in range(32):
            R[hb + m + 32, hb + m] = -1.0
            R[hb + m, hb + m + 32] = 1.0
    return R


def make_diag_mask():
    """[128,128] bf16: 1 where s_idx <= q_idx (valid), else 0."""
    m = np.triu(np.ones((128, 128), dtype=np.float32))
    return m.astype(ml_dtypes.bfloat16)


def prep_in_maps(x, W_qkv, b_qkv, W_out, B, T_, use_qk_bias, use_v_bias=None,
                 n_cores=N_CORES):
    cosT, sinT = make_rope_tables(T_)
    mskP = make_diag_mask()
    rotm = make_rot_matrix()
    xTs = [np.ascontiguousarray(x[b].T) for b in range(B)]
    in_maps = []
    for c in range(n_cores):
        b, hh = c // 2, c % 2
        cols = slice(hh * 512, (hh + 1) * 512)
        m = {
            "xT": xTs[b],
            "wq": np.ascontiguousarray(W_qkv[:, 0 * C:1 * C][:, cols]),
            "wk": np.ascontiguousarray(W_qkv[:, 1 * C:2 * C][:, cols]),
            "wv": np.ascontiguousarray(W_qkv[:, 2 * C:3 * C][:, cols]),
            "wo": np.ascontiguousarray(W_out[cols, :]).astype(
                ml_dtypes.bfloat16),
            "cosT": cosT, "sinT": sinT, "mskP": mskP, "rot": rotm,
        }
        if use_qk_bias:
            m["bq"] = np.ascontiguousarray(
                b_qkv[0 * C:1 * C][cols]).reshape(NP, 128).T.copy()
            m["bk"] = np.ascontiguousarray(
                b_qkv[1 * C:2 * C][cols]).reshape(NP, 128).T.copy()
        in_maps.append(m)
    return in_maps


_CACHE = {}


def _get_program(key):
    if isinstance(key, tuple):
        use_qk_bias = bool(key[2]) if len(key) > 2 else False
    else:
        use_qk_bias = bool(key)
    ck = use_qk_bias
    if ck not in _CACHE:
        _CACHE[ck] = build_program(use_qk_bias)
    return _CACHE[ck]


def kernel(x, W_qkv, b_qkv, W_out, b_out):
    x = np.asarray(x, dtype=np.float32)
    W_qkv = np.asarray(W_qkv, dtype=np.float32)
    b_qkv = np.asarray(b_qkv, dtype=np.float32)
    W_out = np.asarray(W_out, dtype=np.float32)
    b_out = np.asarray(b_out, dtype=np.float32)
    B, T_, C_ = x.shape
    assert (B, T_, C_) == (4, T, C), (B, T_, C_)
    use_qk_bias = bool(np.any(b_qkv[:2 * C]))
    use_v_bias = bool(np.any(b_qkv[2 * C:]))
    nc = _get_program((B, T_, use_qk_bias, use_v_bias))
    in_maps = prep_in_maps(x, W_qkv, b_qkv, W_out, B, T_, use_qk_bias)
    res = run_bass_kernel_spmd(nc, in_maps, list(range(N_CORES)))
    y = np.empty((B, T_, C_), dtype=np.float32)
    for b in range(B):
        y[b] = res.results[2 * b]["outp"]
        y[b] += res.results[2 * b + 1]["outp"]
    extra = b_out.astype(np.float64)
    if use_v_bias:
        extra = extra + b_qkv[2 * C:].astype(np.float64) @ W_out.astype(
            np.float64)
    y += extra.astype(np.float32)[None, None, :]
    return y
